# revision 15
# baseline (speedup 1.0000x reference)
"""Category-specific linear layer (MoE-style routing) on 8 Trainium2 cores.

y[b] = x[b] @ W[cat_ids[b]] + b[cat_ids[b]]
  x: [64, 512, 1024] f32, cat_ids: [64] int, W: [32, 1024, 1024] f32, b: [32, 1024] f32
  y: [64, 512, 1024] f32

Sharding: data-parallel over batch. Core k handles batch elems [8k, 8k+8).
Host gathers W[cat_ids] per core (the routing step), transposes x to [I, T]
layout and casts operands to bf16. Each core runs 8 independent
[512,1024]x[1024,1024] matmuls as 8x8x8 tiled bf16 matmuls (stationary
W-tile [i=128, o=128], moving x^T [i=128, t=512], PSUM [o=128, t=512] f32,
accumulated over 8 i-tiles). Bias is added during the PSUM->SBUF copy on the
vector engine (per-partition scalar), output stored as y^T [O, T] fp16 and
transposed/cast back on host.
"""

from contextlib import ExitStack

import ml_dtypes
import numpy as np

import concourse.bacc as bacc
import concourse.bass as bass
import concourse.mybir as mybir
import concourse.tile as tile
from concourse.bass_utils import run_bass_kernel_spmd

B, T, I, O, C = 64, 512, 1024, 1024, 32
NCORES = 8
NB = B // NCORES          # batch elems per core
PT = 128                  # partition tile
IT = I // PT              # i-tiles (contraction)
OT = O // PT              # o-tiles (output partition)
TN = 512                  # moving free dim == one PSUM bank of f32

BF16 = mybir.dt.bfloat16
F16 = mybir.dt.float16
F32 = mybir.dt.float32

_NC_CACHE = None


def _build_nc():
    global _NC_CACHE
    if _NC_CACHE is not None:
        return _NC_CACHE

    nc = bacc.Bacc("TRN2", target_bir_lowering=False, debug=False,
                   num_devices=NCORES)

    # Host pre-permuted layouts so every DMA is long-contiguous per partition.
    # xt[b, p, it, t] = x[b, t, it*128+p]   (x^T, i split into [it, p])
    xt_d = nc.dram_tensor("xt", [NB, PT, IT, T], BF16, kind="ExternalInput")
    # w[b, p, it, o] = W[cat_ids[b], it*128+p, o]
    w_d = nc.dram_tensor("w", [NB, PT, IT, O], BF16, kind="ExternalInput")
    # bias[p, b*OT+ot] = b[cat_ids[b], ot*128+p]
    bias_d = nc.dram_tensor("bias", [PT, NB * OT], F32, kind="ExternalInput")
    # yt[b, o, t] = y[b, t, o]
    yt_d = nc.dram_tensor("yt", [NB, O, T], F16, kind="ExternalOutput")

    with tile.TileContext(nc) as tc, ExitStack() as ctx:
        xpool = ctx.enter_context(tc.tile_pool(name="xp", bufs=3))
        wpool = ctx.enter_context(tc.tile_pool(name="wp", bufs=3))
        opool = ctx.enter_context(tc.tile_pool(name="op", bufs=8))
        cpool = ctx.enter_context(tc.tile_pool(name="cp", bufs=1))
        pspool = ctx.enter_context(tc.tile_pool(name="ps", bufs=8, space="PSUM"))

        # bias via SWDGE so both HWDGE rings stay free for the data streams
        bias_sb = cpool.tile([PT, NB * OT], F32)
        nc.gpsimd.dma_start(bias_sb[:], bias_d[:])

        # PE warmup: ~3.4us of junk matmuls while the first loads are in
        # flight, so the HAM clock-gate reaches 8/8 (2.4 GHz) before real
        # data arrives. Zero tile so the sim doesn't see uninit reads.
        warm_sb = cpool.tile([PT, TN], BF16)
        nc.vector.memset(warm_sb[:], 0)
        warm_ps = pspool.tile([PT, TN], F32, name="warm_ps", tag="ps")
        for _ in range(5):
            nc.tensor.matmul(warm_ps[:], warm_sb[:, :PT], warm_sb[:],
                             start=True, stop=True)

        # First two batches: per-i-tile chunked loads + i-outer "phase A" so
        # the PE can start as soon as the first (x_i, w_i) chunk pair lands
        # (pipeline fill). Chunked loads cost ~17% DMA throughput (smaller
        # descriptors), so steady-state batches use single whole-tensor
        # loads and the plain o-outer/i-inner order, which profiling shows
        # runs the PE 99% dense.
        NCHUNKED = 2
        IA = IT // 2

        for b in range(NB):
            x_sb = xpool.tile([PT, IT, T], BF16)
            w_sb = wpool.tile([PT, IT, O], BF16)
            # Two parallel load streams: W on the SP HWDGE ring, x on the ACT
            # HWDGE ring. Each ring is FIFO, so splitting the streams roughly
            # doubles fill-phase delivery and keeps batch k+1's data ahead of
            # the PE.
            if b < NCHUNKED:
                # per-i chunks only for the phase-A tiles (early PE start);
                # one bulk DMA for the rest to keep descriptor overhead low
                for i in range(IA):
                    nc.scalar.dma_start(x_sb[:, i, :], xt_d[b, :, i, :])
                    nc.sync.dma_start(w_sb[:, i, :], w_d[b, :, i, :])
                nc.scalar.dma_start(x_sb[:, IA:, :], xt_d[b, :, IA:, :])
                nc.sync.dma_start(w_sb[:, IA:, :], w_d[b, :, IA:, :])
            else:
                nc.scalar.dma_start(x_sb[:], xt_d[b])
                nc.sync.dma_start(w_sb[:], w_d[b])

            def epilogue(o, ps_o):
                y_sb = opool.tile([PT, TN], F16, name=f"y_b{b}o{o}", tag="y")
                nc.vector.tensor_scalar_add(
                    y_sb[:], ps_o[:], bias_sb[:, b * OT + o:b * OT + o + 1])
                # separate HWDGE ring (ACT) so stores don't queue behind loads
                nc.scalar.dma_start(yt_d[b, o * PT:(o + 1) * PT, :], y_sb[:])

            if b < NCHUNKED:
                # phase A: i-outer across all 8 PSUM banks, consumes chunks
                # as they arrive; phase B: o-outer so DVE drains stagger.
                ps = [pspool.tile([PT, TN], F32, name=f"ps_b{b}o{o}", tag="ps")
                      for o in range(OT)]
                for i in range(IA):
                    for o in range(OT):
                        nc.tensor.matmul(
                            ps[o][:],
                            w_sb[:, i, o * PT:(o + 1) * PT],
                            x_sb[:, i, :],
                            start=(i == 0),
                            stop=False,
                        )
                for o in range(OT):
                    for i in range(IA, IT):
                        nc.tensor.matmul(
                            ps[o][:],
                            w_sb[:, i, o * PT:(o + 1) * PT],
                            x_sb[:, i, :],
                            start=False,
                            stop=(i == IT - 1),
                        )
                    epilogue(o, ps[o])
            else:
                for o in range(OT):
                    ps_o = pspool.tile([PT, TN], F32, name=f"ps_b{b}o{o}",
                                       tag="ps")
                    for i in range(IT):
                        nc.tensor.matmul(
                            ps_o[:],
                            w_sb[:, i, o * PT:(o + 1) * PT],
                            x_sb[:, i, :],
                            start=(i == 0),
                            stop=(i == IT - 1),
                        )
                    epilogue(o, ps_o)

    nc.compile()
    _NC_CACHE = nc
    return nc


def _prep_in_maps(x, cat_ids, W, b):
    x = np.asarray(x, dtype=np.float32)
    cat_ids = np.asarray(cat_ids).astype(np.int64)
    W = np.asarray(W, dtype=np.float32)
    b = np.asarray(b, dtype=np.float32)
    assert x.shape == (B, T, I) and cat_ids.shape == (B,)
    assert W.shape == (C, I, O) and b.shape == (C, O)

    # [B, T, I] -> [B, PT, IT, T] bf16  (x^T with i split)
    xt = np.ascontiguousarray(
        x.reshape(B, T, IT, PT).transpose(0, 3, 2, 1)).astype(ml_dtypes.bfloat16)
    Wb = W.astype(ml_dtypes.bfloat16)          # [C, I, O]
    bsel = b[cat_ids]                          # [B, O] f32

    in_maps = []
    for k in range(NCORES):
        sl = slice(k * NB, (k + 1) * NB)
        w_core = Wb[cat_ids[sl]]               # [NB, I, O]
        w_core = np.ascontiguousarray(
            w_core.reshape(NB, IT, PT, O).transpose(0, 2, 1, 3))
        bias_core = np.ascontiguousarray(
            bsel[sl].reshape(NB, OT, PT).transpose(2, 0, 1).reshape(PT, NB * OT))
        in_maps.append({
            "xt": np.ascontiguousarray(xt[sl]),
            "w": w_core,
            "bias": bias_core,
        })
    return in_maps


def run(inputs: dict, trace: bool = False):
    """Returns (y, BassKernelResults)."""
    nc = _build_nc()
    in_maps = _prep_in_maps(**inputs)
    res = run_bass_kernel_spmd(nc, in_maps, core_ids=list(range(NCORES)),
                               trace=trace)
    outs = [r["yt"] for r in res.results]      # each [NB, O, T] fp16
    y = np.concatenate(
        [o.transpose(0, 2, 1).astype(np.float32) for o in outs], axis=0)
    return y, res


def kernel(**inputs) -> np.ndarray:
    y, _ = run(inputs)
    return y


# revision 16
# speedup vs baseline: 1.0086x; 1.0086x over previous
"""Category-specific linear layer (MoE-style routing) on 8 Trainium2 cores.

y[b] = x[b] @ W[cat_ids[b]] + b[cat_ids[b]]
  x: [64, 512, 1024] f32, cat_ids: [64] int, W: [32, 1024, 1024] f32, b: [32, 1024] f32
  y: [64, 512, 1024] f32

Sharding: data-parallel over batch. Core k handles batch elems [8k, 8k+8).
Host gathers W[cat_ids] per core (the routing step), transposes x to [I, T]
layout and casts operands to bf16. Each core runs 8 independent
[512,1024]x[1024,1024] matmuls as 8x8x8 tiled bf16 matmuls (stationary
W-tile [i=128, o=128], moving x^T [i=128, t=512], PSUM [o=128, t=512] f32,
accumulated over 8 i-tiles). Bias is added during the PSUM->SBUF copy on the
vector engine (per-partition scalar), output stored as y^T [O, T] fp16 and
transposed/cast back on host.
"""

from contextlib import ExitStack

import ml_dtypes
import numpy as np

import concourse.bacc as bacc
import concourse.bass as bass
import concourse.mybir as mybir
import concourse.tile as tile
from concourse.bass_utils import run_bass_kernel_spmd

B, T, I, O, C = 64, 512, 1024, 1024, 32
NCORES = 8
NB = B // NCORES          # batch elems per core
PT = 128                  # partition tile
IT = I // PT              # i-tiles (contraction)
OT = O // PT              # o-tiles (output partition)
TN = 512                  # moving free dim == one PSUM bank of f32

BF16 = mybir.dt.bfloat16
F16 = mybir.dt.float16
F32 = mybir.dt.float32

_NC_CACHE = None


def _build_nc():
    global _NC_CACHE
    if _NC_CACHE is not None:
        return _NC_CACHE

    nc = bacc.Bacc("TRN2", target_bir_lowering=False, debug=False,
                   num_devices=NCORES)

    # Host pre-permuted layouts so every DMA is long-contiguous per partition.
    # xt[b, p, it, t] = x[b, t, it*128+p]   (x^T, i split into [it, p])
    xt_d = nc.dram_tensor("xt", [NB, PT, IT, T], BF16, kind="ExternalInput")
    # w[b, p, it, o] = W[cat_ids[b], it*128+p, o]
    w_d = nc.dram_tensor("w", [NB, PT, IT, O], BF16, kind="ExternalInput")
    # bias[p, b*OT+ot] = b[cat_ids[b], ot*128+p]
    bias_d = nc.dram_tensor("bias", [PT, NB * OT], F32, kind="ExternalInput")
    # yt[b, o, t] = y[b, t, o]
    yt_d = nc.dram_tensor("yt", [NB, O, T], F16, kind="ExternalOutput")

    with tile.TileContext(nc) as tc, ExitStack() as ctx:
        xpool = ctx.enter_context(tc.tile_pool(name="xp", bufs=3))
        wpool = ctx.enter_context(tc.tile_pool(name="wp", bufs=3))
        opool = ctx.enter_context(tc.tile_pool(name="op", bufs=8))
        cpool = ctx.enter_context(tc.tile_pool(name="cp", bufs=1))
        pspool = ctx.enter_context(tc.tile_pool(name="ps", bufs=8, space="PSUM"))

        # bias via SWDGE so both HWDGE rings stay free for the data streams
        bias_sb = cpool.tile([PT, NB * OT], F32)
        nc.gpsimd.dma_start(bias_sb[:], bias_d[:])

        # PE warmup: ~3.4us of junk matmuls while the first loads are in
        # flight, so the HAM clock-gate reaches 8/8 (2.4 GHz) before real
        # data arrives. Zero tile so the sim doesn't see uninit reads.
        warm_sb = cpool.tile([PT, TN], BF16)
        nc.vector.memset(warm_sb[:], 0)
        warm_ps = pspool.tile([PT, TN], F32, name="warm_ps", tag="ps")
        for _ in range(9):
            nc.tensor.matmul(warm_ps[:], warm_sb[:, :PT], warm_sb[:],
                             start=True, stop=True)

        # First two batches: per-i-tile chunked loads + i-outer "phase A" so
        # the PE can start as soon as the first (x_i, w_i) chunk pair lands
        # (pipeline fill). Chunked loads cost ~17% DMA throughput (smaller
        # descriptors), so steady-state batches use single whole-tensor
        # loads and the plain o-outer/i-inner order, which profiling shows
        # runs the PE 99% dense.
        NCHUNKED = 2
        IA = IT // 2

        for b in range(NB):
            x_sb = xpool.tile([PT, IT, T], BF16)
            w_sb = wpool.tile([PT, IT, O], BF16)
            # Two parallel load streams: W on the SP HWDGE ring, x on the ACT
            # HWDGE ring. Each ring is FIFO, so splitting the streams roughly
            # doubles fill-phase delivery and keeps batch k+1's data ahead of
            # the PE.
            if b < NCHUNKED:
                # per-i chunks only for the phase-A tiles (early PE start);
                # one bulk DMA for the rest to keep descriptor overhead low
                for i in range(IA):
                    nc.scalar.dma_start(x_sb[:, i, :], xt_d[b, :, i, :])
                    nc.sync.dma_start(w_sb[:, i, :], w_d[b, :, i, :])
                nc.scalar.dma_start(x_sb[:, IA:, :], xt_d[b, :, IA:, :])
                nc.sync.dma_start(w_sb[:, IA:, :], w_d[b, :, IA:, :])
            else:
                nc.scalar.dma_start(x_sb[:], xt_d[b])
                nc.sync.dma_start(w_sb[:], w_d[b])

            def epilogue(o, ps_o):
                y_sb = opool.tile([PT, TN], F16, name=f"y_b{b}o{o}", tag="y")
                nc.vector.tensor_scalar_add(
                    y_sb[:], ps_o[:], bias_sb[:, b * OT + o:b * OT + o + 1])
                # separate HWDGE ring (ACT) so stores don't queue behind loads
                nc.scalar.dma_start(yt_d[b, o * PT:(o + 1) * PT, :], y_sb[:])

            if b < NCHUNKED:
                # phase A: i-outer across all 8 PSUM banks, consumes chunks
                # as they arrive; phase B: o-outer so DVE drains stagger.
                ps = [pspool.tile([PT, TN], F32, name=f"ps_b{b}o{o}", tag="ps")
                      for o in range(OT)]
                for i in range(IA):
                    for o in range(OT):
                        nc.tensor.matmul(
                            ps[o][:],
                            w_sb[:, i, o * PT:(o + 1) * PT],
                            x_sb[:, i, :],
                            start=(i == 0),
                            stop=False,
                        )
                for o in range(OT):
                    for i in range(IA, IT):
                        nc.tensor.matmul(
                            ps[o][:],
                            w_sb[:, i, o * PT:(o + 1) * PT],
                            x_sb[:, i, :],
                            start=False,
                            stop=(i == IT - 1),
                        )
                    epilogue(o, ps[o])
            else:
                for o in range(OT):
                    ps_o = pspool.tile([PT, TN], F32, name=f"ps_b{b}o{o}",
                                       tag="ps")
                    for i in range(IT):
                        nc.tensor.matmul(
                            ps_o[:],
                            w_sb[:, i, o * PT:(o + 1) * PT],
                            x_sb[:, i, :],
                            start=(i == 0),
                            stop=(i == IT - 1),
                        )
                    epilogue(o, ps_o)

    nc.compile()
    _NC_CACHE = nc
    return nc


def _prep_in_maps(x, cat_ids, W, b):
    x = np.asarray(x, dtype=np.float32)
    cat_ids = np.asarray(cat_ids).astype(np.int64)
    W = np.asarray(W, dtype=np.float32)
    b = np.asarray(b, dtype=np.float32)
    assert x.shape == (B, T, I) and cat_ids.shape == (B,)
    assert W.shape == (C, I, O) and b.shape == (C, O)

    # [B, T, I] -> [B, PT, IT, T] bf16  (x^T with i split)
    xt = np.ascontiguousarray(
        x.reshape(B, T, IT, PT).transpose(0, 3, 2, 1)).astype(ml_dtypes.bfloat16)
    Wb = W.astype(ml_dtypes.bfloat16)          # [C, I, O]
    bsel = b[cat_ids]                          # [B, O] f32

    in_maps = []
    for k in range(NCORES):
        sl = slice(k * NB, (k + 1) * NB)
        w_core = Wb[cat_ids[sl]]               # [NB, I, O]
        w_core = np.ascontiguousarray(
            w_core.reshape(NB, IT, PT, O).transpose(0, 2, 1, 3))
        bias_core = np.ascontiguousarray(
            bsel[sl].reshape(NB, OT, PT).transpose(2, 0, 1).reshape(PT, NB * OT))
        in_maps.append({
            "xt": np.ascontiguousarray(xt[sl]),
            "w": w_core,
            "bias": bias_core,
        })
    return in_maps


def run(inputs: dict, trace: bool = False):
    """Returns (y, BassKernelResults)."""
    nc = _build_nc()
    in_maps = _prep_in_maps(**inputs)
    res = run_bass_kernel_spmd(nc, in_maps, core_ids=list(range(NCORES)),
                               trace=trace)
    outs = [r["yt"] for r in res.results]      # each [NB, O, T] fp16
    y = np.concatenate(
        [o.transpose(0, 2, 1).astype(np.float32) for o in outs], axis=0)
    return y, res


def kernel(**inputs) -> np.ndarray:
    y, _ = run(inputs)
    return y


# revision 17
# speedup vs baseline: 1.0154x; 1.0068x over previous
"""Category-specific linear layer (MoE-style routing) on 8 Trainium2 cores.

y[b] = x[b] @ W[cat_ids[b]] + b[cat_ids[b]]
  x: [64, 512, 1024] f32, cat_ids: [64] int, W: [32, 1024, 1024] f32, b: [32, 1024] f32
  y: [64, 512, 1024] f32

Sharding: data-parallel over batch. Core k handles batch elems [8k, 8k+8).
Host gathers W[cat_ids] per core (the routing step), transposes x to [I, T]
layout and casts operands to bf16. Each core runs 8 independent
[512,1024]x[1024,1024] matmuls as 8x8x8 tiled bf16 matmuls (stationary
W-tile [i=128, o=128], moving x^T [i=128, t=512], PSUM [o=128, t=512] f32,
accumulated over 8 i-tiles). Bias is added during the PSUM->SBUF copy on the
vector engine (per-partition scalar), output stored as y^T [O, T] fp16 and
transposed/cast back on host.
"""

from contextlib import ExitStack

import ml_dtypes
import numpy as np

import concourse.bacc as bacc
import concourse.bass as bass
import concourse.mybir as mybir
import concourse.tile as tile
from concourse.bass_utils import run_bass_kernel_spmd

B, T, I, O, C = 64, 512, 1024, 1024, 32
NCORES = 8
NB = B // NCORES          # batch elems per core
PT = 128                  # partition tile
IT = I // PT              # i-tiles (contraction)
OT = O // PT              # o-tiles (output partition)
TN = 512                  # moving free dim == one PSUM bank of f32

BF16 = mybir.dt.bfloat16
F16 = mybir.dt.float16
F32 = mybir.dt.float32

_NC_CACHE = None


def _build_nc():
    global _NC_CACHE
    if _NC_CACHE is not None:
        return _NC_CACHE

    nc = bacc.Bacc("TRN2", target_bir_lowering=False, debug=False,
                   num_devices=NCORES)

    # Host pre-permuted layouts so every DMA is long-contiguous per partition.
    # xt[b, p, it, t] = x[b, t, it*128+p]   (x^T, i split into [it, p])
    xt_d = nc.dram_tensor("xt", [NB, PT, IT, T], BF16, kind="ExternalInput")
    # w[b, p, it, o] = W[cat_ids[b], it*128+p, o]
    w_d = nc.dram_tensor("w", [NB, PT, IT, O], BF16, kind="ExternalInput")
    # bias[p, b*OT+ot] = b[cat_ids[b], ot*128+p]
    bias_d = nc.dram_tensor("bias", [PT, NB * OT], F32, kind="ExternalInput")
    # yt[b, o, t] = y[b, t, o]
    yt_d = nc.dram_tensor("yt", [NB, O, T], F16, kind="ExternalOutput")

    with tile.TileContext(nc) as tc, ExitStack() as ctx:
        xpool = ctx.enter_context(tc.tile_pool(name="xp", bufs=3))
        wpool = ctx.enter_context(tc.tile_pool(name="wp", bufs=3))
        opool = ctx.enter_context(tc.tile_pool(name="op", bufs=8))
        cpool = ctx.enter_context(tc.tile_pool(name="cp", bufs=1))
        pspool = ctx.enter_context(tc.tile_pool(name="ps", bufs=8, space="PSUM"))

        # bias via SWDGE so both HWDGE rings stay free for the data streams
        bias_sb = cpool.tile([PT, NB * OT], F32)
        nc.gpsimd.dma_start(bias_sb[:], bias_d[:])

        # PE warmup: ~3.4us of junk matmuls while the first loads are in
        # flight, so the HAM clock-gate reaches 8/8 (2.4 GHz) before real
        # data arrives. Zero tile so the sim doesn't see uninit reads.
        warm_sb = cpool.tile([PT, TN], BF16)
        nc.vector.memset(warm_sb[:], 0)
        warm_ps = pspool.tile([PT, TN], F32, name="warm_ps", tag="ps")
        for _ in range(9):
            nc.tensor.matmul(warm_ps[:], warm_sb[:, :PT], warm_sb[:],
                             start=True, stop=True)

        # First two batches: per-i-tile chunked loads + i-outer "phase A" so
        # the PE can start as soon as the first (x_i, w_i) chunk pair lands
        # (pipeline fill). Chunked loads cost ~17% DMA throughput (smaller
        # descriptors), so steady-state batches use single whole-tensor
        # loads and the plain o-outer/i-inner order, which profiling shows
        # runs the PE 99% dense.
        NCHUNKED = 2
        IA = IT // 2

        for b in range(NB):
            x_sb = xpool.tile([PT, IT, T], BF16)
            w_sb = wpool.tile([PT, IT, O], BF16)
            # Two parallel load streams: W on the SP HWDGE ring, x on the ACT
            # HWDGE ring. Each ring is FIFO, so splitting the streams roughly
            # doubles fill-phase delivery and keeps batch k+1's data ahead of
            # the PE.
            if b < NCHUNKED:
                # per-i chunks only for the phase-A tiles (early PE start);
                # one bulk DMA for the rest to keep descriptor overhead low
                for i in range(IA):
                    nc.scalar.dma_start(x_sb[:, i, :], xt_d[b, :, i, :])
                    nc.sync.dma_start(w_sb[:, i, :], w_d[b, :, i, :])
                nc.scalar.dma_start(x_sb[:, IA:, :], xt_d[b, :, IA:, :])
                nc.sync.dma_start(w_sb[:, IA:, :], w_d[b, :, IA:, :])
            else:
                nc.scalar.dma_start(x_sb[:], xt_d[b])
                nc.sync.dma_start(w_sb[:], w_d[b])

            def epilogue(o, ps_o):
                y_sb = opool.tile([PT, TN], F16, name=f"y_b{b}o{o}", tag="y")
                nc.vector.tensor_scalar_add(
                    y_sb[:], ps_o[:], bias_sb[:, b * OT + o:b * OT + o + 1])
                # separate HWDGE ring (ACT) so stores don't queue behind loads
                nc.scalar.dma_start(yt_d[b, o * PT:(o + 1) * PT, :], y_sb[:])

            if b < NCHUNKED:
                # phase A: i-outer across all 8 PSUM banks, consumes chunks
                # as they arrive; phase B: o-outer so DVE drains stagger.
                ps = [pspool.tile([PT, TN], F32, name=f"ps_b{b}o{o}", tag="ps")
                      for o in range(OT)]
                for i in range(IA):
                    for o in range(OT):
                        nc.tensor.matmul(
                            ps[o][:],
                            w_sb[:, i, o * PT:(o + 1) * PT],
                            x_sb[:, i, :],
                            start=(i == 0),
                            stop=False,
                        )
                for o in range(OT):
                    for i in range(IA, IT):
                        nc.tensor.matmul(
                            ps[o][:],
                            w_sb[:, i, o * PT:(o + 1) * PT],
                            x_sb[:, i, :],
                            start=False,
                            stop=(i == IT - 1),
                        )
                    epilogue(o, ps[o])
            else:
                for o in range(OT):
                    if b == NB - 1 and o == OT - 1:
                        # Final output tile: two half-width chains so the
                        # last drain+store is half-sized and overlaps the
                        # first half's epilogue (shorter kernel tail).
                        for h in range(2):
                            ps_h = pspool.tile([PT, TN // 2], F32,
                                               name=f"ps_b{b}o{o}h{h}",
                                               tag="ps")
                            hs = slice(h * (TN // 2), (h + 1) * (TN // 2))
                            for i in range(IT):
                                nc.tensor.matmul(
                                    ps_h[:],
                                    w_sb[:, i, o * PT:(o + 1) * PT],
                                    x_sb[:, i, hs],
                                    start=(i == 0),
                                    stop=(i == IT - 1),
                                )
                            y_sb = opool.tile([PT, TN // 2], F16,
                                              name=f"y_b{b}o{o}h{h}", tag="y")
                            nc.vector.tensor_scalar_add(
                                y_sb[:], ps_h[:],
                                bias_sb[:, b * OT + o:b * OT + o + 1])
                            nc.scalar.dma_start(
                                yt_d[b, o * PT:(o + 1) * PT, hs], y_sb[:])
                        continue
                    ps_o = pspool.tile([PT, TN], F32, name=f"ps_b{b}o{o}",
                                       tag="ps")
                    for i in range(IT):
                        nc.tensor.matmul(
                            ps_o[:],
                            w_sb[:, i, o * PT:(o + 1) * PT],
                            x_sb[:, i, :],
                            start=(i == 0),
                            stop=(i == IT - 1),
                        )
                    epilogue(o, ps_o)

    nc.compile()
    _NC_CACHE = nc
    return nc


def _prep_in_maps(x, cat_ids, W, b):
    x = np.asarray(x, dtype=np.float32)
    cat_ids = np.asarray(cat_ids).astype(np.int64)
    W = np.asarray(W, dtype=np.float32)
    b = np.asarray(b, dtype=np.float32)
    assert x.shape == (B, T, I) and cat_ids.shape == (B,)
    assert W.shape == (C, I, O) and b.shape == (C, O)

    # [B, T, I] -> [B, PT, IT, T] bf16  (x^T with i split)
    xt = np.ascontiguousarray(
        x.reshape(B, T, IT, PT).transpose(0, 3, 2, 1)).astype(ml_dtypes.bfloat16)
    Wb = W.astype(ml_dtypes.bfloat16)          # [C, I, O]
    bsel = b[cat_ids]                          # [B, O] f32

    in_maps = []
    for k in range(NCORES):
        sl = slice(k * NB, (k + 1) * NB)
        w_core = Wb[cat_ids[sl]]               # [NB, I, O]
        w_core = np.ascontiguousarray(
            w_core.reshape(NB, IT, PT, O).transpose(0, 2, 1, 3))
        bias_core = np.ascontiguousarray(
            bsel[sl].reshape(NB, OT, PT).transpose(2, 0, 1).reshape(PT, NB * OT))
        in_maps.append({
            "xt": np.ascontiguousarray(xt[sl]),
            "w": w_core,
            "bias": bias_core,
        })
    return in_maps


def run(inputs: dict, trace: bool = False):
    """Returns (y, BassKernelResults)."""
    nc = _build_nc()
    in_maps = _prep_in_maps(**inputs)
    res = run_bass_kernel_spmd(nc, in_maps, core_ids=list(range(NCORES)),
                               trace=trace)
    outs = [r["yt"] for r in res.results]      # each [NB, O, T] fp16
    y = np.concatenate(
        [o.transpose(0, 2, 1).astype(np.float32) for o in outs], axis=0)
    return y, res


def kernel(**inputs) -> np.ndarray:
    y, _ = run(inputs)
    return y
